# revision 24
# baseline (speedup 1.0000x reference)
"""MoE head (router top-2 + per-expert GELU FFN) on 8 TRN2 NeuronCores.

Strategy (sharding hint: expert parallel with top-k dispatch):
  - Host computes the (tiny) router in float64: logits = x @ Wr.T,
    top-2 experts per token, softmax-over-top2 gate weights.
  - Tokens are gathered per expert on the host ("all-to-all dispatch"),
    padded to a fixed capacity C, and each of the 8 cores runs ONE
    expert's FFN over its gathered tokens:
        y_tok = gate * (gelu(x_tok @ W_in[e].T) @ W_out[e])
  - Host scatter-adds the two expert contributions per token.

  Device kernel (SPMD, one program, per-core data):
    mm1: h^T[dhid, tok] = W_in^T-chunks.T @ x^T-chunks  (PE, accumulate K=512)
    gelu: ACT engine, PSUM -> SBUF
    mm2: y[tok, dh] = h^T-chunks.T @ W_out-chunks       (PE, accumulate K=2048)
    gate: DVE per-partition scalar multiply, then DMA out.

  Matmuls run in bf16 by default (~3.5e-3 scale-relative error vs the
  fp32 reference incl. the bf16 output path, well under the 2e-2 gate;
  1.17x the f32r PE rate and half the DMA bytes).  Inputs are cast to
  bf16 and pre-shuffled on the host into partition-major layouts
  ([128, kc, free]) so every input DMA has multi-KB contiguous runs per
  partition -- descriptor-cheap (the old per-slice DMAs cost ~1.5us each,
  128 1KB descriptors).  The 17 input DMAs are round-robined across the
  two HWDGE issue rings (SP, ACT) in first-use order so the PE starts
  ~2us in and the stream hides under compute; SWDGE (gpsimd) measured
  counterproductive (Q7 software descriptor generation).  y is written
  as bf16 (halves output DMA) and the host upcasts + scatter-adds.

  Measured decomposition per loop iteration (cap=1092): PE busy ~62us
  (the bf16 floor for 2x1092x2048x512 MACs at ~0.43ns/col), ~8us PE
  dispatch/ramp overhead, ~5us DMA head/tail exposure, ~6-15us timing-
  loop back-edge cost (barrier + sem resets + I$; branch-prefetch hints
  and staggered_reset measured neutral/negative here and are off).
  Single-shot TimelineSim: 72.4us total, PE 85.5% busy.  PSUM bank limit
  (one matmul output <= 512 fp32 cols) pins the 336-matmul structure.
"""

import os

import numpy as np

P = 128
DH = 512
DHID = 2048
NE = 8
TOPK = 2
KC1 = DH // P  # k-chunks for mm1
KC2 = DHID // P  # k-chunks for mm2
N_CORES = 8

_prog_cache: dict[tuple, object] = {}
LAST_EXEC_NS = None
LAST_RESULTS = None
LAST_CAP = None


def _dtype_mode() -> str:
    return os.environ.get("MOE_DTYPE", "bf16d")  # bf16d | f32r | fp32


def _np_in_dtype(mode: str):
    if mode == "bf16d":
        import ml_dtypes

        return ml_dtypes.bfloat16
    return np.float32


def _groups_of(c: int) -> list[int]:
    """Token groups: each in [256, 512] so the matmul moving dim stays
    >= 256.  All groups except the last are multiples of 128, so every
    group offset is 128-aligned; the last group absorbs any remainder
    (its final mm2 subtile is partial-M)."""
    rem = c % 128
    if rem:
        last = 256 + rem  # in (256, 384)
        body = c - last
    else:
        last = None
        body = c
    gs = []
    left = body
    while left > 640:
        gs.append(512)
        left -= 512
    if left > 512:  # 640+128k leftovers: split into two >=256 chunks
        hi = (left // 256) * 128
        gs.extend([left - hi, hi])
    elif left:
        gs.append(left)
    if last is not None:
        gs.append(last)
    return gs


def program_input_specs(cap: int, mode: str | None = None):
    """(name, shape, np dtype) for the device program's inputs -- used by
    the timing harness to fabricate device-resident dummy inputs."""
    mode = mode or _dtype_mode()
    gcols = -(-cap // P)
    np_in = _np_in_dtype(mode)
    return [
        ("xg", (P, KC1, cap), np_in),
        ("w_in", (P, KC1, DHID), np_in),
        ("w_out", (P, KC2, DH), np_in),
        ("gate", (P, gcols), np.float32),
    ]


def _build_program(
    cap: int, mode: str, reps: int = 1, loop_n: int = 0, loop_scope: str = "all"
):
    """loop_n > 0 wraps the body in a runtime For_i loop (timing only).
    loop_scope="compute" hoists the DMA stage out of the loop;
    loop_scope="loads" loops only the DMA stage."""
    import concourse.mybir as mybir
    import concourse.tile as tile
    from concourse import bacc

    f32 = mybir.dt.float32
    mm_dt = {
        "f32r": mybir.dt.float32r,
        "bf16d": mybir.dt.bfloat16,
        "fp32": mybir.dt.float32,
    }[mode]
    in_dt = mm_dt  # host supplies matmul-dtype bytes directly

    nc = bacc.Bacc(None, target_bir_lowering=False, debug=False)
    xg = nc.declare_dram_parameter("xg", [P, KC1, cap], in_dt, isOutput=False)
    w_in = nc.declare_dram_parameter("w_in", [P, KC1, DHID], in_dt, isOutput=False)
    w_out = nc.declare_dram_parameter("w_out", [P, KC2, DH], in_dt, isOutput=False)
    gcols = -(-cap // P)
    gate = nc.declare_dram_parameter("gate", [P, gcols], f32, isOutput=False)
    y_bf16 = os.environ.get("MOE_YBF16", "1") == "1" and mode == "bf16d"
    y_dt = mybir.dt.bfloat16 if y_bf16 else f32
    y = nc.declare_dram_parameter("y", [cap, DH], y_dt, isOutput=True)

    groups_env = os.environ.get("MOE_GROUPS")
    if groups_env:
        groups = [int(g) for g in groups_env.split(",")]
        assert sum(groups) == cap, (groups, cap)
    else:
        groups = _groups_of(cap)
    gelu = mybir.ActivationFunctionType.Gelu

    with tile.TileContext(nc) as tc:
        with (
            # bufs=2: in the For_i timing loop, iteration i+1's input DMAs
            # write the other buffer, dodging the WAR wait on iteration i's
            # last reads (real single-shot runs have no such hazard).
            tc.tile_pool(
                name="persist",
                bufs=int(
                    os.environ.get(
                        "MOE_INBUF", "2" if mode == "bf16d" and cap <= 2304 else "1"
                    )
                ),
            ) as persist,
            tc.tile_pool(
                name="hbuf",
                bufs=int(os.environ.get("MOE_HBUF", "2" if cap <= 2304 else "1")),
            ) as hbuf,
            tc.tile_pool(name="ybuf", bufs=int(os.environ.get("MOE_YBUF", "4"))) as ybuf,
            tc.tile_pool(name="ps1", bufs=int(os.environ.get("MOE_PS1", "5")), space="PSUM") as ps1,
            tc.tile_pool(name="ps2", bufs=int(os.environ.get("MOE_PS2", "3")), space="PSUM") as ps2,
        ):
            state = {}
            # Round-robin DMA issue across independent paths.  SWDGE
            # (gpsimd) generates descriptors in software on Q7 -- slow for
            # 128-descriptor DMAs -- so default to the two HWDGE rings.
            issuers = {
                "ss": [nc.sync, nc.scalar],
                "ssg": [nc.sync, nc.scalar, nc.gpsimd],
                "s": [nc.sync],
            }[os.environ.get("MOE_ISSUERS", "ss")]

            def emit_loads():
                xg_r = persist.tile([P, KC1, cap], in_dt, tag="xg_r")
                w_in_r = persist.tile([P, KC1, DHID], in_dt, tag="w_in_r")
                w_out_r = persist.tile([P, KC2, DH], in_dt, tag="w_out_r")
                gate_sb = persist.tile([P, gcols], f32, tag="gate_sb")
                H2 = DHID // 2
                gran = os.environ.get("MOE_DMAGRAN", "fine")
                dmas = []
                # first-use order: mm1 g0 d0 needs w_in[:, kc, 0:128] and
                # xg[:, kc, g0] for all kc -- land those first.
                if gran == "v2":  # tiny starters + earlier w_in h1
                    g0 = groups[0]
                    for kc in range(KC1):
                        # starters: exactly the first matmul chain's operands
                        dmas.append((w_in_r[:, kc, 0:P], w_in[:, kc, 0:P]))
                        dmas.append((xg_r[:, kc, 0:g0], xg[:, kc, 0:g0]))
                    for kc in range(KC1):
                        dmas.append((w_in_r[:, kc, P:H2], w_in[:, kc, P:H2]))
                        dmas.append((w_in_r[:, kc, H2:DHID], w_in[:, kc, H2:DHID]))
                    for kc in range(KC1):
                        dmas.append((xg_r[:, kc, g0:cap], xg[:, kc, g0:cap]))
                    for kc4 in range(KC2 // 4):
                        dmas.append(
                            (
                                w_out_r[:, kc4 * 4 : (kc4 + 1) * 4, :],
                                w_out[:, kc4 * 4 : (kc4 + 1) * 4, :],
                            )
                        )
                elif gran == "fine":  # 17 DMAs
                    for kc in range(KC1):
                        dmas.append((w_in_r[:, kc, 0:H2], w_in[:, kc, 0:H2]))
                        dmas.append((xg_r[:, kc, :], xg[:, kc, :]))
                    for kc in range(KC1):
                        dmas.append((w_in_r[:, kc, H2:DHID], w_in[:, kc, H2:DHID]))
                    for kc4 in range(KC2 // 4):
                        dmas.append(
                            (
                                w_out_r[:, kc4 * 4 : (kc4 + 1) * 4, :],
                                w_out[:, kc4 * 4 : (kc4 + 1) * 4, :],
                            )
                        )
                elif gran == "med":  # 11 DMAs
                    for kc in range(KC1):
                        dmas.append((w_in_r[:, kc, :], w_in[:, kc, :]))
                        dmas.append((xg_r[:, kc, :], xg[:, kc, :]))
                    for h in range(2):
                        dmas.append(
                            (
                                w_out_r[:, h * 8 : (h + 1) * 8, :],
                                w_out[:, h * 8 : (h + 1) * 8, :],
                            )
                        )
                else:  # coarse: 6 DMAs
                    for h in range(2):
                        dmas.append(
                            (w_in_r[:, h * 2 : (h + 1) * 2, :], w_in[:, h * 2 : (h + 1) * 2, :])
                        )
                        dmas.append(
                            (xg_r[:, h * 2 : (h + 1) * 2, :], xg[:, h * 2 : (h + 1) * 2, :])
                        )
                    dmas.append((w_out_r[:, :, :], w_out[:, :, :]))
                dmas.append((gate_sb[:, :], gate[:, :]))
                for i, (dst, src) in enumerate(dmas):
                    issuers[i % len(issuers)].dma_start(out=dst, in_=src)
                state.update(
                    xg_r=xg_r, w_in_r=w_in_r, w_out_r=w_out_r, gate_sb=gate_sb
                )

            def emit_compute():
                # ---- mm1 -> gelu -> mm2 -> gate -> out, per token-group ----
                # Phase order is staggered (mm1 g0, mm1 g1, mm2 g0, mm1 g2,
                # mm2 g1, mm2 g2) so the first mm2 starts ~2 mm1-phases into
                # the kernel, giving the w_out DMA stream time to land
                # without stalling the PE. Needs 2 live h tiles (hbuf=2).
                xg_r, w_in_r = state["xg_r"], state["w_in_r"]
                w_out_r, gate_sb = state["w_out_r"], state["gate_sb"]
                offs = []
                off = 0
                for gsz in groups:
                    offs.append(off)
                    off += gsz
                h_tiles = {}

                def mm1_phase(gi):
                    gsz, off = groups[gi], offs[gi]
                    h_r = hbuf.tile([P, KC2, gsz], mm_dt, tag="h_r")
                    h_tiles[gi] = h_r
                    for d in range(KC2):
                        ps = ps1.tile([P, gsz], f32, tag="p1")
                        for kc in range(KC1):
                            nc.tensor.matmul(
                                ps,
                                w_in_r[:, kc, d * P : (d + 1) * P],
                                xg_r[:, kc, off : off + gsz],
                                start=(kc == 0),
                                stop=(kc == KC1 - 1),
                            )
                        nc.scalar.activation(h_r[:, d, :], ps, gelu)

                def mm2_phase(gi):
                    gsz, off = groups[gi], offs[gi]
                    h_r = h_tiles.pop(gi)
                    for s in range(-(-gsz // P)):
                        m = min(P, gsz - s * P)  # last subtile may be partial
                        pt = ps2.tile([P, DH], f32, tag="p2")
                        for d in range(KC2):
                            nc.tensor.matmul(
                                pt[:m, :],
                                h_r[:, d, s * P : s * P + m],
                                w_out_r[:, d, :],
                                start=(d == 0),
                                stop=(d == KC2 - 1),
                            )
                        tok0 = off + s * P
                        y_sb = ybuf.tile([P, DH], y_dt, tag="y_sb")
                        nc.vector.tensor_scalar_mul(
                            y_sb[:m, :], pt[:m, :], gate_sb[:m, tok0 // P : tok0 // P + 1]
                        )
                        issuers[(tok0 // P) % min(len(issuers), 2)].dma_start(
                            out=y[tok0 : tok0 + m, :], in_=y_sb[:m, :]
                        )

                # Default non-staggered: with the coarse 2-ring DMA stream,
                # w_out lands ~10us in, so delaying mm2 buys nothing and
                # only defers the output stream (measured ~5us worse).
                n_g = len(groups)
                if n_g == 1 or os.environ.get("MOE_STAGGER", "0") != "1":
                    for gi in range(n_g):
                        mm1_phase(gi)
                        mm2_phase(gi)
                else:
                    mm1_phase(0)
                    mm1_phase(1)
                    for gi in range(2, n_g):
                        mm2_phase(gi - 2)
                        mm1_phase(gi)
                    mm2_phase(n_g - 2)
                    mm2_phase(n_g - 1)

            # Timing-loop mechanics: the PE body is ~680 instructions (~3
            # IRAM blocks), so the back-edge branch I$-misses without a
            # prefetch hint (~4us/iteration); staggered_reset replaces the
            # drain + two all-engine barriers (~2us) with overlapped
            # semaphore resets.
            loop_kw = {}
            if loop_n:
                # Branch-prefetch hints measured neutral-to-negative here
                # (hint_cnd register writes outweigh the I$ save) -- off.
                hints = os.environ.get("MOE_LOOPHINT", "")
                if hints:
                    loop_kw["hint_engines"] = tuple(
                        getattr(mybir.EngineType, h) for h in hints.split(",")
                    )
                if os.environ.get("MOE_STAGRESET", "0") == "1":
                    loop_kw["staggered_reset"] = True
            if loop_n and loop_scope == "compute":
                emit_loads()
                with tc.For_i(0, loop_n, 1, **loop_kw):
                    for _rep in range(reps):
                        emit_compute()
            elif loop_n and loop_scope == "loads":
                with tc.For_i(0, loop_n, 1, **loop_kw):
                    for _rep in range(reps):
                        emit_loads()
                emit_compute()
            elif loop_n:
                with tc.For_i(0, loop_n, 1, **loop_kw):
                    for _rep in range(reps):
                        emit_loads()
                        emit_compute()
            else:
                for _rep in range(reps):
                    emit_loads()
                    emit_compute()

    nc.compile()
    return nc


def _get_program(cap: int, mode: str):
    key = (cap, mode)
    if key not in _prog_cache:
        _prog_cache[key] = _build_program(cap, mode)
    return _prog_cache[key]


def kernel(x, Wr, W_in, W_out):
    global LAST_EXEC_NS, LAST_RESULTS, LAST_CAP
    from concourse.bass_utils import run_bass_kernel_spmd

    x = np.ascontiguousarray(np.asarray(x), dtype=np.float32)
    Wr = np.asarray(Wr, dtype=np.float32)
    W_in = np.asarray(W_in, dtype=np.float32)
    W_out = np.asarray(W_out, dtype=np.float32)
    T = x.shape[0]

    # ---- host router (fp64: strictly more accurate than the fp32 ref) ----
    logits = x.astype(np.float64) @ Wr.astype(np.float64).T  # (T, NE)
    part = np.argpartition(-logits, TOPK - 1, axis=1)[:, :TOPK]
    vals = np.take_along_axis(logits, part, axis=1)
    order = np.argsort(-vals, axis=1, kind="stable")
    idx = np.take_along_axis(part, order, axis=1)  # (T, 2) desc
    ar = np.arange(T)
    v1 = logits[ar, idx[:, 0]]
    v2 = logits[ar, idx[:, 1]]
    e2 = np.exp(v2 - v1)
    w1 = (1.0 / (1.0 + e2)).astype(np.float32)
    w2 = (e2 / (1.0 + e2)).astype(np.float32)

    tok_lists, gate_lists = [], []
    for e in range(NE):
        s1 = np.nonzero(idx[:, 0] == e)[0]
        s2 = np.nonzero(idx[:, 1] == e)[0]
        tok_lists.append(np.concatenate([s1, s2]))
        gate_lists.append(np.concatenate([w1[s1], w2[s2]]))
    max_count = max(len(t) for t in tok_lists)

    cap_env = os.environ.get("MOE_CAP")
    cap = int(cap_env) if cap_env else -(-max_count // 4) * 4  # exact-ish
    if max_count > cap:
        cap = -(-max_count // 4) * 4
    cap = max(cap, 384)
    mode = _dtype_mode()
    LAST_CAP = cap
    nc = _get_program(cap, mode)

    np_in = _np_in_dtype(mode)
    x_c = x.astype(np_in)
    W_in_c = W_in.astype(np_in)
    W_out_c = W_out.astype(np_in)
    gcols = -(-cap // P)
    in_maps = []
    for e in range(NE):
        toks = tok_lists[e]
        n = len(toks)
        # partition-major layouts: [128, kc, free] with row kc*128+p -> (p, kc)
        xg = np.zeros((P, KC1, cap), np_in)
        xg[:, :, :n] = x_c[toks].T.reshape(KC1, P, n).transpose(1, 0, 2)
        w_in_h = W_in_c[e].T.reshape(KC1, P, DHID).transpose(1, 0, 2)
        w_out_h = W_out_c[e].reshape(KC2, P, DH).transpose(1, 0, 2)
        g = np.zeros((gcols * P,), np.float32)
        g[:n] = gate_lists[e]
        in_maps.append(
            {
                "xg": xg,
                "w_in": np.ascontiguousarray(w_in_h),
                "w_out": np.ascontiguousarray(w_out_h),
                "gate": np.ascontiguousarray(g.reshape(gcols, P).T),
            }
        )

    trace = os.environ.get("MOE_TRACE", "0") == "1"
    res = run_bass_kernel_spmd(
        nc,
        in_maps,
        list(range(N_CORES)),
        trace=trace,
        trace_cores=list(range(N_CORES)) if trace else None,
    )
    LAST_EXEC_NS = res.exec_time_ns
    LAST_RESULTS = res

    out = np.zeros((T, DH), np.float32)
    for e in range(NE):
        toks = tok_lists[e]
        if len(toks):
            out[toks] += res.results[e]["y"][: len(toks)].astype(np.float32)
    return out


# revision 28
# speedup vs baseline: 1.0347x; 1.0347x over previous
"""MoE head (router top-2 + per-expert GELU FFN) on 8 TRN2 NeuronCores.

Strategy (sharding hint: expert parallel with top-k dispatch):
  - Host computes the (tiny) router in float64: logits = x @ Wr.T,
    top-2 experts per token, softmax-over-top2 gate weights.
  - Tokens are gathered per expert on the host ("all-to-all dispatch"),
    padded to a fixed capacity C, and each of the 8 cores runs ONE
    expert's FFN over its gathered tokens:
        y_tok = gate * (gelu(x_tok @ W_in[e].T) @ W_out[e])
  - Host scatter-adds the two expert contributions per token.

  Device kernel (SPMD, one program, per-core data):
    mm1: h^T[dhid, tok] = W_in^T-chunks.T @ x^T-chunks  (PE, accumulate K=512)
    gelu: ACT engine, PSUM -> SBUF
    mm2: y[tok, dh] = h^T-chunks.T @ W_out-chunks       (PE, accumulate K=2048)
    gate: DVE per-partition scalar multiply, then DMA out.

  Matmuls run in bf16 by default (~3.5e-3 scale-relative error vs the
  fp32 reference incl. the bf16 output path, well under the 2e-2 gate;
  1.17x the f32r PE rate and half the DMA bytes).  Inputs are cast to
  bf16 and pre-shuffled on the host into partition-major layouts
  ([128, kc, free]) so every input DMA has multi-KB contiguous runs per
  partition -- descriptor-cheap (the old per-slice DMAs cost ~1.5us each,
  128 1KB descriptors).  The 17 input DMAs are round-robined across the
  two HWDGE issue rings (SP, ACT) in first-use order so the PE starts
  ~2us in and the stream hides under compute; SWDGE (gpsimd) measured
  counterproductive (Q7 software descriptor generation).  y is written
  as bf16 (halves output DMA) and the host upcasts + scatter-adds.

  Measured decomposition per loop iteration (cap=1092): PE busy ~62us
  (the bf16 floor for 2x1092x2048x512 MACs at ~0.43ns/col), ~8us PE
  dispatch/ramp overhead, ~5us DMA head/tail exposure, ~6-15us timing-
  loop back-edge cost (barrier + sem resets + I$; branch-prefetch hints
  and staggered_reset measured neutral/negative here and are off).
  Single-shot TimelineSim: 72.4us total, PE 85.5% busy.  PSUM bank limit
  (one matmul output <= 512 fp32 cols) pins the 336-matmul structure.
"""

import os

import numpy as np

P = 128
DH = 512
DHID = 2048
NE = 8
TOPK = 2
KC1 = DH // P  # k-chunks for mm1
KC2 = DHID // P  # k-chunks for mm2
N_CORES = 8

_prog_cache: dict[tuple, object] = {}
LAST_EXEC_NS = None
LAST_RESULTS = None
LAST_CAP = None


def _dtype_mode() -> str:
    return os.environ.get("MOE_DTYPE", "bf16d")  # bf16d | f32r | fp32


def _np_in_dtype(mode: str):
    if mode == "bf16d":
        import ml_dtypes

        return ml_dtypes.bfloat16
    return np.float32


def _groups_of(c: int) -> list[int]:
    """Token groups: each in [256, 512] so the matmul moving dim stays
    >= 256.  All groups except the last are multiples of 128, so every
    group offset is 128-aligned; the last group absorbs any remainder
    (its final mm2 subtile is partial-M)."""
    rem = c % 128
    if rem:
        last = 256 + rem  # in (256, 384)
        body = c - last
    else:
        last = None
        body = c
    gs = []
    left = body
    while left > 640:
        gs.append(512)
        left -= 512
    if left > 512:  # 640+128k leftovers: split into two >=256 chunks
        hi = (left // 256) * 128
        gs.extend([left - hi, hi])
    elif left:
        gs.append(left)
    if last is not None:
        gs.append(last)
    return gs


def program_input_specs(cap: int, mode: str | None = None):
    """(name, shape, np dtype) for the device program's inputs -- used by
    the timing harness to fabricate device-resident dummy inputs."""
    mode = mode or _dtype_mode()
    gcols = -(-cap // P)
    np_in = _np_in_dtype(mode)
    return [
        ("xg", (P, KC1, cap), np_in),
        ("w_in", (P, KC1, DHID), np_in),
        ("w_out", (P, KC2, DH), np_in),
        ("gate", (P, gcols), np.float32),
    ]


def _build_program(
    cap: int, mode: str, reps: int = 1, loop_n: int = 0, loop_scope: str = "all"
):
    """loop_n > 0 wraps the body in a runtime For_i loop (timing only).
    loop_scope="compute" hoists the DMA stage out of the loop;
    loop_scope="loads" loops only the DMA stage."""
    import concourse.mybir as mybir
    import concourse.tile as tile
    from concourse import bacc

    f32 = mybir.dt.float32
    mm_dt = {
        "f32r": mybir.dt.float32r,
        "bf16d": mybir.dt.bfloat16,
        "fp32": mybir.dt.float32,
    }[mode]
    in_dt = mm_dt  # host supplies matmul-dtype bytes directly

    nc = bacc.Bacc(None, target_bir_lowering=False, debug=False)
    xg = nc.declare_dram_parameter("xg", [P, KC1, cap], in_dt, isOutput=False)
    w_in = nc.declare_dram_parameter("w_in", [P, KC1, DHID], in_dt, isOutput=False)
    w_out = nc.declare_dram_parameter("w_out", [P, KC2, DH], in_dt, isOutput=False)
    gcols = -(-cap // P)
    gate = nc.declare_dram_parameter("gate", [P, gcols], f32, isOutput=False)
    y_bf16 = os.environ.get("MOE_YBF16", "1") == "1" and mode == "bf16d"
    y_dt = mybir.dt.bfloat16 if y_bf16 else f32
    y = nc.declare_dram_parameter("y", [cap, DH], y_dt, isOutput=True)

    groups_env = os.environ.get("MOE_GROUPS")
    if groups_env:
        groups = [int(g) for g in groups_env.split(",")]
        assert sum(groups) == cap, (groups, cap)
    else:
        groups = _groups_of(cap)
    gelu = mybir.ActivationFunctionType.Gelu

    with tile.TileContext(nc) as tc:
        with (
            # bufs=2: in the For_i timing loop, iteration i+1's input DMAs
            # write the other buffer, dodging the WAR wait on iteration i's
            # last reads (real single-shot runs have no such hazard).
            tc.tile_pool(
                name="persist",
                bufs=int(
                    os.environ.get(
                        "MOE_INBUF", "2" if mode == "bf16d" and cap <= 2304 else "1"
                    )
                ),
            ) as persist,
            tc.tile_pool(
                name="hbuf",
                bufs=int(os.environ.get("MOE_HBUF", "2" if cap <= 2304 else "1")),
            ) as hbuf,
            tc.tile_pool(name="ybuf", bufs=int(os.environ.get("MOE_YBUF", "4"))) as ybuf,
            tc.tile_pool(name="ps1", bufs=int(os.environ.get("MOE_PS1", "5")), space="PSUM") as ps1,
            tc.tile_pool(name="ps2", bufs=int(os.environ.get("MOE_PS2", "3")), space="PSUM") as ps2,
        ):
            state = {}
            # Round-robin DMA issue across independent paths.  SWDGE
            # (gpsimd) generates descriptors in software on Q7 -- slow for
            # 128-descriptor DMAs -- so default to the two HWDGE rings.
            issuers = {
                "ss": [nc.sync, nc.scalar],
                "ssg": [nc.sync, nc.scalar, nc.gpsimd],
                "s": [nc.sync],
            }[os.environ.get("MOE_ISSUERS", "ss")]

            def emit_loads():
                xg_r = persist.tile([P, KC1, cap], in_dt, tag="xg_r")
                w_in_r = persist.tile([P, KC1, DHID], in_dt, tag="w_in_r")
                w_out_r = persist.tile([P, KC2, DH], in_dt, tag="w_out_r")
                gate_sb = persist.tile([P, gcols], f32, tag="gate_sb")
                H2 = DHID // 2
                gran = os.environ.get("MOE_DMAGRAN", "fine")
                dmas = []
                # first-use order: mm1 g0 d0 needs w_in[:, kc, 0:128] and
                # xg[:, kc, g0] for all kc -- land those first.
                if gran == "v2":  # tiny starters + earlier w_in h1
                    g0 = groups[0]
                    for kc in range(KC1):
                        # starters: exactly the first matmul chain's operands
                        dmas.append((w_in_r[:, kc, 0:P], w_in[:, kc, 0:P]))
                        dmas.append((xg_r[:, kc, 0:g0], xg[:, kc, 0:g0]))
                    for kc in range(KC1):
                        dmas.append((w_in_r[:, kc, P:H2], w_in[:, kc, P:H2]))
                        dmas.append((w_in_r[:, kc, H2:DHID], w_in[:, kc, H2:DHID]))
                    for kc in range(KC1):
                        dmas.append((xg_r[:, kc, g0:cap], xg[:, kc, g0:cap]))
                    for kc4 in range(KC2 // 4):
                        dmas.append(
                            (
                                w_out_r[:, kc4 * 4 : (kc4 + 1) * 4, :],
                                w_out[:, kc4 * 4 : (kc4 + 1) * 4, :],
                            )
                        )
                elif gran == "fine":  # 17 DMAs (adding more costs more in
                    # per-DMA fixed overhead than earlier landings save --
                    # sim: 72.4us at 17 DMAs vs 73.7/74.3 for 21-DMA splits)
                    for kc in range(KC1):
                        dmas.append((w_in_r[:, kc, 0:H2], w_in[:, kc, 0:H2]))
                        dmas.append((xg_r[:, kc, :], xg[:, kc, :]))
                    for kc in range(KC1):
                        dmas.append((w_in_r[:, kc, H2:DHID], w_in[:, kc, H2:DHID]))
                    for kc4 in range(KC2 // 4):
                        dmas.append(
                            (
                                w_out_r[:, kc4 * 4 : (kc4 + 1) * 4, :],
                                w_out[:, kc4 * 4 : (kc4 + 1) * 4, :],
                            )
                        )
                    dmas.append((gate_sb[:, :], gate[:, :]))
                elif gran == "med":  # 11 DMAs
                    for kc in range(KC1):
                        dmas.append((w_in_r[:, kc, :], w_in[:, kc, :]))
                        dmas.append((xg_r[:, kc, :], xg[:, kc, :]))
                    for h in range(2):
                        dmas.append(
                            (
                                w_out_r[:, h * 8 : (h + 1) * 8, :],
                                w_out[:, h * 8 : (h + 1) * 8, :],
                            )
                        )
                else:  # coarse: 6 DMAs
                    for h in range(2):
                        dmas.append(
                            (w_in_r[:, h * 2 : (h + 1) * 2, :], w_in[:, h * 2 : (h + 1) * 2, :])
                        )
                        dmas.append(
                            (xg_r[:, h * 2 : (h + 1) * 2, :], xg[:, h * 2 : (h + 1) * 2, :])
                        )
                    dmas.append((w_out_r[:, :, :], w_out[:, :, :]))
                if gran != "fine":  # fine emits gate in first-use position
                    dmas.append((gate_sb[:, :], gate[:, :]))
                for i, (dst, src) in enumerate(dmas):
                    issuers[i % len(issuers)].dma_start(out=dst, in_=src)
                state.update(
                    xg_r=xg_r, w_in_r=w_in_r, w_out_r=w_out_r, gate_sb=gate_sb
                )

            def emit_compute():
                # ---- mm1 -> gelu -> mm2 -> gate -> out, per token-group ----
                # Phase order is staggered (mm1 g0, mm1 g1, mm2 g0, mm1 g2,
                # mm2 g1, mm2 g2) so the first mm2 starts ~2 mm1-phases into
                # the kernel, giving the w_out DMA stream time to land
                # without stalling the PE. Needs 2 live h tiles (hbuf=2).
                xg_r, w_in_r = state["xg_r"], state["w_in_r"]
                w_out_r, gate_sb = state["w_out_r"], state["gate_sb"]
                offs = []
                off = 0
                for gsz in groups:
                    offs.append(off)
                    off += gsz
                h_tiles = {}

                def mm1_phase(gi):
                    gsz, off = groups[gi], offs[gi]
                    h_r = hbuf.tile([P, KC2, gsz], mm_dt, tag="h_r")
                    h_tiles[gi] = h_r
                    for d in range(KC2):
                        ps = ps1.tile([P, gsz], f32, tag="p1")
                        for kc in range(KC1):
                            nc.tensor.matmul(
                                ps,
                                w_in_r[:, kc, d * P : (d + 1) * P],
                                xg_r[:, kc, off : off + gsz],
                                start=(kc == 0),
                                stop=(kc == KC1 - 1),
                            )
                        nc.scalar.activation(h_r[:, d, :], ps, gelu)

                def mm2_phase(gi):
                    gsz, off = groups[gi], offs[gi]
                    h_r = h_tiles.pop(gi)
                    for s in range(-(-gsz // P)):
                        m = min(P, gsz - s * P)  # last subtile may be partial
                        pt = ps2.tile([P, DH], f32, tag="p2")
                        for d in range(KC2):
                            nc.tensor.matmul(
                                pt[:m, :],
                                h_r[:, d, s * P : s * P + m],
                                w_out_r[:, d, :],
                                start=(d == 0),
                                stop=(d == KC2 - 1),
                            )
                        tok0 = off + s * P
                        y_sb = ybuf.tile([P, DH], y_dt, tag="y_sb")
                        nc.vector.tensor_scalar_mul(
                            y_sb[:m, :], pt[:m, :], gate_sb[:m, tok0 // P : tok0 // P + 1]
                        )
                        issuers[(tok0 // P) % min(len(issuers), 2)].dma_start(
                            out=y[tok0 : tok0 + m, :], in_=y_sb[:m, :]
                        )

                # Default non-staggered: with the coarse 2-ring DMA stream,
                # w_out lands ~10us in, so delaying mm2 buys nothing and
                # only defers the output stream (measured ~5us worse).
                n_g = len(groups)
                if n_g == 1 or os.environ.get("MOE_STAGGER", "0") != "1":
                    for gi in range(n_g):
                        mm1_phase(gi)
                        mm2_phase(gi)
                else:
                    mm1_phase(0)
                    mm1_phase(1)
                    for gi in range(2, n_g):
                        mm2_phase(gi - 2)
                        mm1_phase(gi)
                    mm2_phase(n_g - 2)
                    mm2_phase(n_g - 1)

            # Timing-loop mechanics: the PE body is ~680 instructions (~3
            # IRAM blocks), so the back-edge branch I$-misses without a
            # prefetch hint (~4us/iteration); staggered_reset replaces the
            # drain + two all-engine barriers (~2us) with overlapped
            # semaphore resets.
            loop_kw = {}
            if loop_n:
                # Branch-prefetch hints measured neutral-to-negative here
                # (hint_cnd register writes outweigh the I$ save) -- off.
                hints = os.environ.get("MOE_LOOPHINT", "")
                if hints:
                    loop_kw["hint_engines"] = tuple(
                        getattr(mybir.EngineType, h) for h in hints.split(",")
                    )
                if os.environ.get("MOE_STAGRESET", "0") == "1":
                    loop_kw["staggered_reset"] = True
            if loop_n and loop_scope == "compute":
                emit_loads()
                with tc.For_i(0, loop_n, 1, **loop_kw):
                    for _rep in range(reps):
                        emit_compute()
            elif loop_n and loop_scope == "loads":
                with tc.For_i(0, loop_n, 1, **loop_kw):
                    for _rep in range(reps):
                        emit_loads()
                emit_compute()
            elif loop_n:
                with tc.For_i(0, loop_n, 1, **loop_kw):
                    for _rep in range(reps):
                        emit_loads()
                        emit_compute()
            else:
                for _rep in range(reps):
                    emit_loads()
                    emit_compute()

    nc.compile()
    return nc


def _get_program(cap: int, mode: str):
    key = (cap, mode)
    if key not in _prog_cache:
        _prog_cache[key] = _build_program(cap, mode)
    return _prog_cache[key]


def kernel(x, Wr, W_in, W_out):
    global LAST_EXEC_NS, LAST_RESULTS, LAST_CAP
    from concourse.bass_utils import run_bass_kernel_spmd

    x = np.ascontiguousarray(np.asarray(x), dtype=np.float32)
    Wr = np.asarray(Wr, dtype=np.float32)
    W_in = np.asarray(W_in, dtype=np.float32)
    W_out = np.asarray(W_out, dtype=np.float32)
    T = x.shape[0]

    # ---- host router (fp64: strictly more accurate than the fp32 ref) ----
    logits = x.astype(np.float64) @ Wr.astype(np.float64).T  # (T, NE)
    part = np.argpartition(-logits, TOPK - 1, axis=1)[:, :TOPK]
    vals = np.take_along_axis(logits, part, axis=1)
    order = np.argsort(-vals, axis=1, kind="stable")
    idx = np.take_along_axis(part, order, axis=1)  # (T, 2) desc
    ar = np.arange(T)
    v1 = logits[ar, idx[:, 0]]
    v2 = logits[ar, idx[:, 1]]
    e2 = np.exp(v2 - v1)
    w1 = (1.0 / (1.0 + e2)).astype(np.float32)
    w2 = (e2 / (1.0 + e2)).astype(np.float32)

    tok_lists, gate_lists = [], []
    for e in range(NE):
        s1 = np.nonzero(idx[:, 0] == e)[0]
        s2 = np.nonzero(idx[:, 1] == e)[0]
        tok_lists.append(np.concatenate([s1, s2]))
        gate_lists.append(np.concatenate([w1[s1], w2[s2]]))
    max_count = max(len(t) for t in tok_lists)

    cap_env = os.environ.get("MOE_CAP")
    cap = int(cap_env) if cap_env else -(-max_count // 4) * 4  # exact-ish
    if max_count > cap:
        cap = -(-max_count // 4) * 4
    cap = max(cap, 384)
    mode = _dtype_mode()
    LAST_CAP = cap
    nc = _get_program(cap, mode)

    np_in = _np_in_dtype(mode)
    x_c = x.astype(np_in)
    W_in_c = W_in.astype(np_in)
    W_out_c = W_out.astype(np_in)
    gcols = -(-cap // P)
    in_maps = []
    for e in range(NE):
        toks = tok_lists[e]
        n = len(toks)
        # partition-major layouts: [128, kc, free] with row kc*128+p -> (p, kc)
        xg = np.zeros((P, KC1, cap), np_in)
        xg[:, :, :n] = x_c[toks].T.reshape(KC1, P, n).transpose(1, 0, 2)
        w_in_h = W_in_c[e].T.reshape(KC1, P, DHID).transpose(1, 0, 2)
        w_out_h = W_out_c[e].reshape(KC2, P, DH).transpose(1, 0, 2)
        g = np.zeros((gcols * P,), np.float32)
        g[:n] = gate_lists[e]
        in_maps.append(
            {
                "xg": xg,
                "w_in": np.ascontiguousarray(w_in_h),
                "w_out": np.ascontiguousarray(w_out_h),
                "gate": np.ascontiguousarray(g.reshape(gcols, P).T),
            }
        )

    trace = os.environ.get("MOE_TRACE", "0") == "1"
    res = run_bass_kernel_spmd(
        nc,
        in_maps,
        list(range(N_CORES)),
        trace=trace,
        trace_cores=list(range(N_CORES)) if trace else None,
    )
    LAST_EXEC_NS = res.exec_time_ns
    LAST_RESULTS = res

    out = np.zeros((T, DH), np.float32)
    for e in range(NE):
        toks = tok_lists[e]
        if len(toks):
            out[toks] += res.results[e]["y"][: len(toks)].astype(np.float32)
    return out
